# revision 31
# baseline (speedup 1.0000x reference)
"""Trainium2 Bass kernel for nn_DKAModule (dynamic-kernel attention).

Per core (data-parallel over B*n = 8192 tokens -> 1024/core + 10-token halo),
all matmuls bf16 (1 cycle/row on the PE), PSUM fp32:

  x_projT = W_in @ x^T                   (PE)
  per head h:
    S_r   = band matmuls over transposed token windows (PE; transposes
            4-packed per PSUM tile, one Act copy per pack)
    csv_r = S_r * cbv_r                  (Act evacuates S to bf16 SBUF,
            DVE bf16 multiply; cbv = alpha*V[r,d]*c_r[n] host-side)
    o_h   = sum_r csv_r + static conv
            h7: PE diag-matmuls + ident r-sum, single Act evac
            h0-h6: DVE tensor_scalar+tensor_add shift-MACs + add-tree
  out     = o^T @ W_out^T                (PE); b_out added on host

All DRAM tensors are host-packed into the exact SBUF layout so every DMA
is one large contiguous descriptor per partition (small descriptors gate
DMA to a fraction of peak). dma_starts are ordered by criticality across
the sync/gpsimd issue queues. The PE is kept dense (HAM clock gate) via a
one-head software pipeline: band matmuls of head i-1 run while head i
transposes.
"""
import sys
import types

import numpy as np
import ml_dtypes

BF = ml_dtypes.bfloat16

KS = [3, 3, 7, 7, 11, 11, 21, 21]
H, DM, DH, R, B, N = 8, 1024, 128, 4, 2, 4096
NC = 8
TPC = B * N // NC
PAD = 10
TH = TPC + 2 * PAD  # 1044
S1C = TH // 3  # 348

PE_SHIFTS = {5: 11, 6: 21, 7: 21}  # head -> leading static shifts on PE
PE_STATIC_HEADS = tuple(sorted(PE_SHIFTS))
HEAD_ORDER = (7, 4, 6, 3, 5, 2, 0, 1)
PASS_B = (0, 1)  # heads joined into the out-projection in a second pass
FIRST_MS = HEAD_ORDER[:2]
REST_MS = HEAD_ORDER[2:]
NF = len(FIRST_MS)

_MODULE_CACHE = {}


def _install_ntff_hook_shim():
    """This image's antenv lacks axon_hooks; provide it so profiling works."""
    if "antenv.axon_hooks" in sys.modules:
        return
    try:
        from trn_agent_boot.trn_boot import _ntff_profile_via_ctypes

        hook = _ntff_profile_via_ctypes("/opt/axon/libaxon_pjrt.so")
    except Exception:
        hook = None
    mod = types.ModuleType("antenv.axon_hooks")
    mod.get_axon_ntff_profile_hook = lambda: hook
    mod.set_axon_ntff_profile_hook = lambda h: None
    sys.modules["antenv.axon_hooks"] = mod


def _split_multi_waits(nc, mybir):
    """walrus codegen allows a single sync-wait per instruction; hoist
    extras onto a chain of single-wait NoOps on the same engine."""
    for f in nc.m.functions:
        for blk in f.blocks:
            new_insts = []
            for inst in blk.instructions:
                si = getattr(inst, "sync_info", None)
                ow = list(si.on_wait) if si and si.on_wait else []
                if len(ow) >= 2:
                    for i, w in enumerate(ow[:-1]):
                        new_insts.append(
                            mybir.InstNoOp(
                                name=f"{inst.name}-wn{i}",
                                ins=[],
                                outs=[],
                                engine=inst.engine,
                                sync_info=mybir.SyncInfo(on_wait=[w], on_update=[]),
                            )
                        )
                    inst.sync_info = mybir.SyncInfo(
                        on_wait=[ow[-1]],
                        on_update=list(si.on_update) if si.on_update else [],
                    )
                new_insts.append(inst)
            blk.instructions = new_insts


def _window_params(h):
    k = KS[h]
    p = k // 2
    W = 128 - 2 * p
    nw = -(-TPC // W)
    return k, p, W, nw


def _band_offsets():
    offs = {}
    o = 0
    for h in range(H):
        k, p, W, nw = _window_params(h)
        offs[h] = o
        o += R * W
    return offs, o


def _w_col(i, m):
    """Column of stationary block (i, m) inside the packed w tile."""
    if m in FIRST_MS:
        return i * (NF * 128) + FIRST_MS.index(m) * 128
    return H * NF * 128 + i * ((H - NF) * 128) + REST_MS.index(m) * 128


def _x_col(i, ci):
    """Column of chunk ci of contraction slice i inside the packed x tile.
    Layout: [chunk0: i-major][chunk1: i-major][chunk2: i-major]."""
    return ci * H * S1C + i * S1C


def _build_module():
    import concourse.bass as bass
    import concourse.tile as tile
    from concourse import mybir

    f32 = mybir.dt.float32
    bf16 = mybir.dt.bfloat16
    MUL = mybir.AluOpType.mult

    nc = bass.Bass(trn_type="TRN2")

    band_offs, band_tot = _band_offsets()

    # ---- DRAM I/O (host-packed layouts) ----
    xpk_d = nc.dram_tensor("xpk", [128, H * TH], bf16, kind="ExternalInput")
    wpk_d = nc.dram_tensor("wpk", [128, H * DM], bf16, kind="ExternalInput")
    wopk_d = nc.dram_tensor("wopk", [128, H * DM], bf16, kind="ExternalInput")
    cbv_d = nc.dram_tensor("cbv", [128, H * R * TPC], bf16, kind="ExternalInput")
    band_d = nc.dram_tensor("band", [128, band_tot], bf16, kind="ExternalInput")
    gd_d = {
        h: nc.dram_tensor(f"gdiag{h}", [DH, KS[h] * DH], bf16, kind="ExternalInput")
        for h in PE_STATIC_HEADS
    }
    gvec_d = nc.dram_tensor("gvec", [DH, H * 21], f32, kind="ExternalInput")
    b_in_d = nc.dram_tensor("b_in", [128, H], f32, kind="ExternalInput")
    ident_d = nc.dram_tensor("ident", [128, 128], bf16, kind="ExternalInput")
    out_d = nc.dram_tensor("out", [TPC, DM], bf16, kind="ExternalOutput")

    cbv4 = cbv_d.rearrange("p (hh r t) -> p hh r t", hh=H, r=R)
    NFC = H * NF * 128  # size of the "first" w block

    with tile.TileContext(nc) as tc:
        with tc.tile_pool(name="const", bufs=1) as pc:
            xp_sb = [pc.tile([DH, TH], bf16, name=f"xp{m}") for m in range(H)]
            o_sb = [pc.tile([DH, TPC], bf16, name=f"o{h}") for h in range(H)]
            ident_sb = pc.tile([128, 128], bf16, name="ident_sb")
            b_in_sb = pc.tile([128, H], f32, name="b_in_sb")
            gvec_sb = pc.tile([DH, H * 21], f32, name="gvec_sb")
            band_sb = pc.tile([128, band_tot], bf16, name="band_sb")
            gd_sb = {
                h: pc.tile([DH, KS[h] * DH], bf16, name=f"gd{h}")
                for h in PE_STATIC_HEADS
            }
            w_all = pc.tile([128, H * DM], bf16, name="w_all")
            x_all = pc.tile([128, H * TH], bf16, name="x_all")
            wo_sb = pc.tile([128, H * DM], bf16, name="wo_all")
            wo3 = wo_sb.rearrange("p (i m) -> p i m", i=H)

            # --- DMAs by criticality; every transfer is contiguous ---
            nc.sync.dma_start(out=w_all[:, :NFC], in_=wpk_d[:, :NFC])
            HS1 = H * S1C // 2
            nc.sync.dma_start(out=x_all[:, :HS1], in_=xpk_d[:, :HS1])
            nc.sync.dma_start(out=x_all[:, HS1 : H * S1C], in_=xpk_d[:, HS1 : H * S1C])
            nc.gpsimd.dma_start(out=ident_sb, in_=ident_d[:, :])
            nc.gpsimd.dma_start(out=b_in_sb, in_=b_in_d[:, :])
            nc.sync.dma_start(out=w_all[:, NFC:], in_=wpk_d[:, NFC:])
            nc.gpsimd.dma_start(out=band_sb, in_=band_d[:, :])
            # rest of x, split by stage-1 chunk so chunk 1 lands earlier
            nc.sync.dma_start(
                out=x_all[:, H * S1C : 2 * H * S1C],
                in_=xpk_d[:, H * S1C : 2 * H * S1C],
            )
            nc.sync.dma_start(
                out=x_all[:, 2 * H * S1C :], in_=xpk_d[:, 2 * H * S1C :]
            )
            for h in PE_STATIC_HEADS:
                nc.gpsimd.dma_start(out=gd_sb[h], in_=gd_d[h][:, :])
            nc.gpsimd.dma_start(out=gvec_sb, in_=gvec_d[:, :])

            # ---------------- stages 1+2 (head 7 overlaps stage 1) --------
            with tc.tile_pool(name="s2cbv", bufs=3) as p2c, tc.tile_pool(
                name="s2csv", bufs=3
            ) as p2v, tc.tile_pool(name="s2s", bufs=4) as p2s, tc.tile_pool(
                name="s2x", bufs=7
            ) as p2x, tc.tile_pool(name="s2acc", bufs=3) as p2a, tc.tile_pool(
                name="s2t", bufs=6
            ) as p2t:
                s2psum = tc.tile_pool(name="ps2tp", bufs=2, space="PSUM")
                pptp = s2psum.__enter__()
                s2psum_s = tc.tile_pool(name="ps2s", bufs=2, space="PSUM")
                ppss = s2psum_s.__enter__()
                s2psum_o = None
                ppso = None
                csv3_of = {}
                cbv_sb_of = {}
                xtd_of = {}

                def fetch_cbv(h, eng):
                    cbv_sb = p2c.tile([DH, R * TPC], bf16, name=f"cbv{h}", tag="cbv")
                    eng.dma_start(
                        out=cbv_sb.rearrange("p (r t) -> p r t", r=R),
                        in_=cbv4[:, h, :, :],
                    )
                    cbv_sb_of[h] = cbv_sb

                fetch_cbv(HEAD_ORDER[0], nc.sync)
                fetch_cbv(HEAD_ORDER[1], nc.sync)

                def transp_phase(h):
                    k, p, W, nw = _window_params(h)
                    xph = xp_sb[h]
                    xtds = []
                    for g0 in range(0, nw, 4):
                        gn = min(4, nw - g0)
                        tp4 = pptp.tile([128, 512], bf16, name="tp4", tag="tp4")
                        xtd4 = p2x.tile([128, 512], bf16, name="xtd4", tag="xtd4")
                        maxcnt = 0
                        for gi in range(gn):
                            b = g0 + gi
                            off = PAD - p + b * W
                            cnt = min(128, TH - off)
                            maxcnt = max(maxcnt, cnt)
                            nc.tensor.transpose(
                                tp4[:cnt, gi * 128 : gi * 128 + 128],
                                xph[:, off : off + cnt],
                                ident_sb,
                            )
                        nc.scalar.copy(
                            xtd4[:maxcnt, : gn * 128], tp4[:maxcnt, : gn * 128]
                        )
                        for gi in range(gn):
                            xtds.append((xtd4, gi))
                    xtd_of[h] = xtds

                def band_phase(h):
                    k, p, W, nw = _window_params(h)
                    bo = band_offs[h]
                    b2 = band_sb[:, bo : bo + R * W]
                    cbv3 = cbv_sb_of.pop(h).rearrange("p (r t) -> p r t", r=R)
                    csv_sb = p2v.tile([DH, R * TPC], bf16, name=f"csv{h}", tag="csv")
                    csv3 = csv_sb.rearrange("p (r t) -> p r t", r=R)
                    csv3_of[h] = csv3
                    xtds = xtd_of.pop(h)
                    for q0 in range(0, nw, 2):
                        qn = min(2, nw - q0)
                        # halves bank-aligned at 0/512 (a matmul output must
                        # not cross a 2KB PSUM bank boundary)
                        ps_s = ppss.tile([128, 1024], f32, name="ps_s", tag="ps_s")
                        s_sb = p2s.tile([128, 1024], bf16, name="s_sb", tag="s_sb")
                        muls = []
                        for qi in range(qn):
                            b = q0 + qi
                            off = PAD - p + b * W
                            cnt = min(128, TH - off)
                            n_out = min(W, TPC - b * W)
                            xtd4, gi = xtds[b]
                            st = xtd4[:cnt, gi * 128 : gi * 128 + 128]
                            if n_out == W:
                                nc.tensor.matmul(
                                    ps_s[:, qi * 512 : qi * 512 + R * W],
                                    st,
                                    b2[:cnt, :],
                                    start=True,
                                    stop=True,
                                )
                            else:
                                band3 = b2.rearrange("p (r w) -> p r w", r=R)
                                nc.tensor.matmul(
                                    ps_s[:, qi * 512 : qi * 512 + R * n_out],
                                    st,
                                    band3[:cnt, :, :n_out],
                                    start=True,
                                    stop=True,
                                )
                            muls.append((qi, b, n_out))
                        if h in PE_STATIC_HEADS:
                            # shorter chain into the PE diag stage: DVE
                            # multiplies straight from PSUM
                            for qi, b, n_out in muls:
                                t0 = b * W
                                nc.vector.tensor_mul(
                                    csv3[:, :, t0 : t0 + n_out],
                                    ps_s[
                                        :, qi * 512 : qi * 512 + R * n_out
                                    ].rearrange("p (r w) -> p r w", r=R),
                                    cbv3[:, :, t0 : t0 + n_out],
                                )
                        else:
                            # one Act evacuation for both halves, then bf16
                            # DVE multiplies (keeps DVE off the PSUM path)
                            hi_end = muls[-1][0] * 512 + R * muls[-1][2]
                            nc.scalar.copy(s_sb[:, :hi_end], ps_s[:, :hi_end])
                            for qi, b, n_out in muls:
                                t0 = b * W
                                nc.vector.tensor_mul(
                                    csv3[:, :, t0 : t0 + n_out],
                                    s_sb[
                                        :, qi * 512 : qi * 512 + R * n_out
                                    ].rearrange("p (r w) -> p r w", r=R),
                                    cbv3[:, :, t0 : t0 + n_out],
                                )

                def stage2b_chunks(h):
                    k, p, W, nw = _window_params(h)
                    xph = xp_sb[h]
                    npe = PE_SHIFTS.get(h, 0)
                    HC = TPC // 2
                    chunks = []
                    if npe > 0:
                        def pe_chunk(c0):
                            csv3 = csv3_of[h]
                            ps_o = ppso.tile([128, 512], f32, name="ps_o", tag="ps_o")
                            nmm = R + npe
                            idx = 0
                            for rr in range(R):
                                nc.tensor.matmul(
                                    ps_o,
                                    ident_sb,
                                    csv3[:, rr, c0 : c0 + 512],
                                    start=(idx == 0),
                                    stop=(idx == nmm - 1),
                                )
                                idx += 1
                            for j in range(npe):
                                o0 = c0 + j - p + PAD
                                nc.tensor.matmul(
                                    ps_o,
                                    gd_sb[h][:, j * DH : (j + 1) * DH],
                                    xph[:, o0 : o0 + 512],
                                    start=(idx == 0),
                                    stop=(idx == nmm - 1),
                                )
                                idx += 1
                            nc.scalar.copy(o_sb[h][:, c0 : c0 + 512], ps_o)
                        chunks += [lambda c0=c0: pe_chunk(c0) for c0 in (0, 512)]

                    if npe < k:
                        sacc = p2a.tile([DH, TPC], bf16, name="sacc", tag="sacc")

                        def dve_chunk(c0):
                            sl = slice(c0, c0 + HC)
                            for jj, j in enumerate(range(npe, k)):
                                sh = PAD + j - p + c0
                                gcol = gvec_sb[:, h * 21 + j : h * 21 + j + 1]
                                if jj == 0:
                                    nc.vector.tensor_scalar(
                                        out=sacc[:, sl],
                                        in0=xph[:, sh : sh + HC],
                                        scalar1=gcol,
                                        scalar2=None,
                                        op0=MUL,
                                    )
                                else:
                                    tmp = p2t.tile(
                                        [DH, HC], bf16, name="tmp", tag="tmp"
                                    )
                                    nc.vector.tensor_scalar(
                                        out=tmp,
                                        in0=xph[:, sh : sh + HC],
                                        scalar1=gcol,
                                        scalar2=None,
                                        op0=MUL,
                                    )
                                    nc.vector.tensor_add(
                                        sacc[:, sl], sacc[:, sl], tmp
                                    )
                            if npe > 0:
                                # PE already produced csv-sum + its shifts
                                nc.vector.tensor_add(
                                    o_sb[h][:, sl], o_sb[h][:, sl], sacc[:, sl]
                                )
                                return
                            csv3 = csv3_of[h]
                            t01 = p2t.tile([DH, HC], bf16, name="t01", tag="tmp")
                            nc.vector.tensor_add(t01, csv3[:, 0, sl], csv3[:, 1, sl])
                            t23 = p2t.tile([DH, HC], bf16, name="t23", tag="tmp")
                            nc.vector.tensor_add(t23, csv3[:, 2, sl], csv3[:, 3, sl])
                            nc.vector.tensor_add(t01, t01, t23)
                            nc.vector.tensor_add(o_sb[h][:, sl], t01, sacc[:, sl])

                        chunks += [lambda c0=c0: dve_chunk(c0) for c0 in (0, HC)]
                    return chunks

                # ---- stage 1 with head-7 stage-2a interleaved ----
                with tc.tile_pool(name="ps1", bufs=2, space="PSUM") as pp1:
                    for ci in range(3):
                        for mi, m in enumerate(HEAD_ORDER):
                            ps = pp1.tile([128, S1C], f32, name="ps_xp", tag="ps_xp")
                            for i in range(H):
                                nc.tensor.matmul(
                                    ps,
                                    w_all[:, _w_col(i, m) : _w_col(i, m) + 128],
                                    x_all[
                                        :, _x_col(i, ci) : _x_col(i, ci) + S1C
                                    ],
                                    start=(i == 0),
                                    stop=(i == H - 1),
                                )
                            nc.scalar.activation(
                                out=xp_sb[m][:, ci * S1C : (ci + 1) * S1C],
                                in_=ps,
                                func=mybir.ActivationFunctionType.Identity,
                                bias=b_in_sb[:, m : m + 1],
                                scale=1.0,
                            )
                            if ci == 2 and mi == 0:
                                transp_phase(HEAD_ORDER[0])
                            elif ci == 2 and mi == 2:
                                band_phase(HEAD_ORDER[0])
                                fetch_cbv(HEAD_ORDER[2], nc.gpsimd)

                s2psum_o = tc.tile_pool(name="ps2o", bufs=2, space="PSUM")
                ppso = s2psum_o.__enter__()

                pending = []
                deferred_b = []
                for fn in stage2b_chunks(HEAD_ORDER[0]):
                    pending.append(fn)
                transp_phase(HEAD_ORDER[1])
                for hi in range(1, H):
                    h = HEAD_ORDER[hi]
                    band_phase(h)
                    if hi + 1 < H:
                        hn = HEAD_ORDER[hi + 1]
                        if hi + 2 < H:
                            fetch_cbv(
                                HEAD_ORDER[hi + 2],
                                nc.sync if hi % 2 == 0 else nc.gpsimd,
                            )
                        transp_phase(hn)
                    if hi == 2:
                        nc.gpsimd.dma_start(out=wo_sb, in_=wopk_d[:, :])
                    for fn in stage2b_chunks(h):
                        if h in PASS_B:
                            deferred_b.append(fn)
                        else:
                            pending.append(fn)
                    for _ in range(2):
                        if pending:
                            pending.pop(0)()
                for fn in pending:
                    fn()
                # stage-2 PSUM pools closed; PASS_B heads' DVE statics still
                # to run (no PSUM needed) - they overlap out-proj pass A
                s2psum_o.__exit__(None, None, None)
                s2psum_s.__exit__(None, None, None)
                s2psum.__exit__(None, None, None)

                # ---------------- stage 4: out projection ----------------
                PASS_A = [i for i in HEAD_ORDER if i not in PASS_B]
                part_sb = [
                    pc.tile([128, DM], bf16, name=f"part{t}")
                    for t in range(TPC // 128)
                ]
                with tc.tile_pool(name="s4o", bufs=3) as p4o, tc.tile_pool(
                    name="ps4", bufs=4, space="PSUM"
                ) as pp4:
                    # pass A: heads done early; runs while DVE finishes the
                    # last two heads' static convs
                    for t in range(TPC // 128):
                        for e0 in (0, 512):
                            ps = pp4.tile(
                                [128, 512], f32, name="ps_out", tag="ps_out"
                            )
                            for ii, i in enumerate(PASS_A):
                                nc.tensor.matmul(
                                    ps,
                                    o_sb[i][:, t * 128 : (t + 1) * 128],
                                    wo3[:, i, e0 : e0 + 512],
                                    start=(ii == 0),
                                    stop=(ii == len(PASS_A) - 1),
                                )
                            nc.scalar.copy(part_sb[t][:, e0 : e0 + 512], ps)
                    for fn in deferred_b:
                        fn()
                    # pass B: late heads' contribution + DVE add of partial
                    for t in range(TPC // 128):
                        ot = p4o.tile([128, DM], bf16, name="out_sb", tag="out_sb")
                        for e0 in (0, 512):
                            ps = pp4.tile(
                                [128, 512], f32, name="ps_out", tag="ps_out"
                            )
                            for ii, i in enumerate(PASS_B):
                                nc.tensor.matmul(
                                    ps,
                                    o_sb[i][:, t * 128 : (t + 1) * 128],
                                    wo3[:, i, e0 : e0 + 512],
                                    start=(ii == 0),
                                    stop=(ii == len(PASS_B) - 1),
                                )
                            nc.vector.tensor_add(
                                ot[:, e0 : e0 + 512],
                                ps,
                                part_sb[t][:, e0 : e0 + 512],
                            )
                        eng = nc.gpsimd if (t % 2 == 0) else nc.sync
                        eng.dma_start(out=out_d[t * 128 : (t + 1) * 128, :], in_=ot)

    _split_multi_waits(nc, mybir)
    return nc


def _host_prep(inputs):
    x = np.ascontiguousarray(np.asarray(inputs["x"], dtype=np.float32))
    W_in = np.asarray(inputs["W_in"], dtype=np.float32)
    b_in = np.asarray(inputs["b_in"], dtype=np.float32)
    W_out = np.asarray(inputs["W_out"], dtype=np.float32)
    b_out = np.asarray(inputs["b_out"], dtype=np.float32)
    Wc = np.asarray(inputs["Wc"], dtype=np.float32)
    A = np.asarray(inputs["A"], dtype=np.float32)
    V = np.asarray(inputs["V"], dtype=np.float32)
    base = np.asarray(inputs["base"], dtype=np.float32)
    alphas = np.asarray(inputs["alphas"], dtype=np.float32)

    alpha = 1.0 / (1.0 + np.exp(-alphas))

    W_inT = np.ascontiguousarray(W_in.T)
    W_outT = np.ascontiguousarray(W_out.T)
    Wc_aug = np.zeros((DM, H * R), dtype=np.float32)
    c_bias = np.zeros((H * R,), dtype=np.float32)
    for h in range(H):
        Wc_aug[:, R * h : R * h + R] = W_inT[:, h * DH : (h + 1) * DH] @ Wc[h]
        c_bias[R * h : R * h + R] = b_in[h * DH : (h + 1) * DH] @ Wc[h]

    # pack W_in into the SBUF layout (see _w_col)
    wpk = np.zeros((128, H * DM), dtype=np.float32)
    for i in range(H):
        for m in range(H):
            wpk[:, _w_col(i, m) : _w_col(i, m) + 128] = W_inT[
                i * 128 : (i + 1) * 128, m * 128 : (m + 1) * 128
            ]
    wopk = np.zeros((128, H * DM), dtype=np.float32)
    for i in range(H):
        wopk[:, i * DM : (i + 1) * DM] = W_outT[i * 128 : (i + 1) * 128, :]

    band_offs, band_tot = _band_offsets()
    band_all = np.zeros((128, band_tot), dtype=np.float32)
    for h in range(H):
        k, p, W, nw = _window_params(h)
        t_in = np.arange(128)[:, None]
        t_out = np.arange(W)[None, :]
        delta = t_in - t_out
        mask = (delta >= 0) & (delta < k)
        dc = np.clip(delta, 0, k - 1)
        band = np.zeros((128, R, W), dtype=np.float32)
        for rr in range(R):
            band[:, rr, :] = np.where(mask, A[h, rr][dc], 0.0)
        band_all[:, band_offs[h] : band_offs[h] + R * W] = band.reshape(128, R * W)

    prep = {
        "wpk": wpk.astype(BF),
        "wopk": wopk.astype(BF),
        "b_in": np.ascontiguousarray(b_in.reshape(H, 128).T),
        "ident": np.eye(128, dtype=BF),
        "band": band_all.astype(BF),
    }

    for h in PE_STATIC_HEADS:
        k = KS[h]
        gd = np.zeros((DH, k, DH), dtype=np.float32)
        g = (1.0 - alpha[h]) * base[h, :k]
        dd = np.arange(DH)
        gd[dd, :, dd] = g.T[dd]
        prep[f"gdiag{h}"] = gd.reshape(DH, k * DH).astype(BF)

    gvec = np.zeros((DH, H, 21), dtype=np.float32)
    for h in range(H):
        k = KS[h]
        gvec[:, h, :k] = ((1.0 - alpha[h]) * base[h, :k]).T
    prep["gvec"] = gvec.reshape(DH, H * 21).copy()

    xT_slices = []
    cbv_slices = []
    per_b = NC // B
    for c in range(NC):
        bb = c // per_b
        s = (c % per_b) * TPC
        sl = np.zeros((TH, DM), dtype=np.float32)
        lo, hi = s - PAD, s + TPC + PAD
        clo, chi = max(lo, 0), min(hi, N)
        sl[clo - lo : chi - lo] = x[bb, clo:chi]
        slT = sl.T  # (DM, TH)
        xpk = np.empty((128, H * TH), dtype=np.float32)
        for i in range(H):
            blk = slT[i * 128 : (i + 1) * 128]
            for ci in range(3):
                xpk[:, _x_col(i, ci) : _x_col(i, ci) + S1C] = blk[
                    :, ci * S1C : (ci + 1) * S1C
                ]
        xT_slices.append(xpk.astype(BF))
        cc = sl[PAD : PAD + TPC] @ Wc_aug + c_bias[None, :]
        cc3 = cc.reshape(TPC, H, R)
        cbv = np.empty((128, H, R, TPC), dtype=np.float32)
        for h in range(H):
            for rr in range(R):
                cbv[:, h, rr, :] = alpha[h] * np.outer(V[h, rr], cc3[:, h, rr])
        cbv_slices.append(
            np.ascontiguousarray(cbv.reshape(128, H * R * TPC)).astype(BF)
        )
    return prep, xT_slices, cbv_slices, b_out


def _run(inputs, trace=False, **kwargs):
    _install_ntff_hook_shim()
    from concourse.bass_utils import run_bass_kernel_spmd

    if "mod" not in _MODULE_CACHE:
        _MODULE_CACHE["mod"] = _build_module()
    nc = _MODULE_CACHE["mod"]

    prep, xT_slices, cbv_slices, b_out = _host_prep(inputs)
    in_maps = []
    for c in range(NC):
        m = dict(prep)
        m["xpk"] = xT_slices[c]
        m["cbv"] = cbv_slices[c]
        in_maps.append(m)

    res = run_bass_kernel_spmd(
        nc, in_maps, core_ids=list(range(NC)), trace=trace, **kwargs
    )
    outs = [np.asarray(res.results[c]["out"], dtype=np.float32) for c in range(NC)]
    full = np.concatenate(outs, axis=0).reshape(B, N, DM)
    full += b_out[None, None, :]
    return full, res


def kernel(**inputs) -> np.ndarray:
    return _run(inputs)[0]


# revision 32
# speedup vs baseline: 1.1822x; 1.1822x over previous
"""Trainium2 Bass kernel for nn_DKAModule (dynamic-kernel attention).

Per core (data-parallel over B*n = 8192 tokens -> 1024/core + 10-token halo),
all matmuls bf16 (1 cycle/row on the PE), PSUM fp32:

  x_projT = W_in @ x^T                   (PE)
  per head h:
    S_r   = band matmuls over transposed token windows (PE; transposes
            4-packed per PSUM tile, one Act copy per pack)
    csv_r = S_r * cbv_r                  (Act evacuates S to bf16 SBUF,
            DVE bf16 multiply; cbv = alpha*V[r,d]*c_r[n] host-side)
    o_h   = sum_r csv_r + static conv
            h7: PE diag-matmuls + ident r-sum, single Act evac
            h0-h6: DVE tensor_scalar+tensor_add shift-MACs + add-tree
  out     = o^T @ W_out^T                (PE); b_out added on host

All DRAM tensors are host-packed into the exact SBUF layout so every DMA
is one large contiguous descriptor per partition (small descriptors gate
DMA to a fraction of peak). dma_starts are ordered by criticality across
the sync/gpsimd issue queues. The PE is kept dense (HAM clock gate) via a
one-head software pipeline: band matmuls of head i-1 run while head i
transposes.
"""
import sys
import types

import numpy as np
import ml_dtypes

BF = ml_dtypes.bfloat16

KS = [3, 3, 7, 7, 11, 11, 21, 21]
H, DM, DH, R, B, N = 8, 1024, 128, 4, 2, 4096
NC = 8
TPC = B * N // NC
PAD = 10
TH = TPC + 2 * PAD  # 1044
S1C = TH // 3  # 348

PE_SHIFTS = {5: 11, 6: 21, 7: 21}  # head -> leading static shifts on PE
PE_STATIC_HEADS = tuple(sorted(PE_SHIFTS))
HEAD_ORDER = (7, 4, 6, 3, 5, 2, 0, 1)
PASS_B = (0, 1)  # heads joined into the out-projection in a second pass
FIRST_MS = HEAD_ORDER[:3]
REST_MS = HEAD_ORDER[3:]
NF = len(FIRST_MS)

_MODULE_CACHE = {}


def _install_ntff_hook_shim():
    """This image's antenv lacks axon_hooks; provide it so profiling works."""
    if "antenv.axon_hooks" in sys.modules:
        return
    try:
        from trn_agent_boot.trn_boot import _ntff_profile_via_ctypes

        hook = _ntff_profile_via_ctypes("/opt/axon/libaxon_pjrt.so")
    except Exception:
        hook = None
    mod = types.ModuleType("antenv.axon_hooks")
    mod.get_axon_ntff_profile_hook = lambda: hook
    mod.set_axon_ntff_profile_hook = lambda h: None
    sys.modules["antenv.axon_hooks"] = mod


def _split_multi_waits(nc, mybir):
    """walrus codegen allows a single sync-wait per instruction; hoist
    extras onto a chain of single-wait NoOps on the same engine."""
    for f in nc.m.functions:
        for blk in f.blocks:
            new_insts = []
            for inst in blk.instructions:
                si = getattr(inst, "sync_info", None)
                ow = list(si.on_wait) if si and si.on_wait else []
                if len(ow) >= 2:
                    for i, w in enumerate(ow[:-1]):
                        new_insts.append(
                            mybir.InstNoOp(
                                name=f"{inst.name}-wn{i}",
                                ins=[],
                                outs=[],
                                engine=inst.engine,
                                sync_info=mybir.SyncInfo(on_wait=[w], on_update=[]),
                            )
                        )
                    inst.sync_info = mybir.SyncInfo(
                        on_wait=[ow[-1]],
                        on_update=list(si.on_update) if si.on_update else [],
                    )
                new_insts.append(inst)
            blk.instructions = new_insts


def _window_params(h):
    k = KS[h]
    p = k // 2
    W = 128 - 2 * p
    nw = -(-TPC // W)
    return k, p, W, nw


def _band_offsets():
    offs = {}
    o = 0
    for h in range(H):
        k, p, W, nw = _window_params(h)
        offs[h] = o
        o += R * W
    return offs, o


def _w_col(i, m):
    """Column of stationary block (i, m) inside the packed w tile."""
    if m in FIRST_MS:
        return i * (NF * 128) + FIRST_MS.index(m) * 128
    return H * NF * 128 + i * ((H - NF) * 128) + REST_MS.index(m) * 128


def _x_col(i, ci):
    """Column of chunk ci of contraction slice i inside the packed x tile.
    Layout: [chunk0: i-major][chunk1: i-major][chunk2: i-major]."""
    return ci * H * S1C + i * S1C


def _build_module():
    import concourse.bass as bass
    import concourse.tile as tile
    from concourse import mybir

    f32 = mybir.dt.float32
    bf16 = mybir.dt.bfloat16
    MUL = mybir.AluOpType.mult

    nc = bass.Bass(trn_type="TRN2")

    band_offs, band_tot = _band_offsets()

    # ---- DRAM I/O (host-packed layouts) ----
    xpk_d = nc.dram_tensor("xpk", [128, H * TH], bf16, kind="ExternalInput")
    wpk_d = nc.dram_tensor("wpk", [128, H * DM], bf16, kind="ExternalInput")
    wopk_d = nc.dram_tensor("wopk", [128, H * DM], bf16, kind="ExternalInput")
    cbv_d = nc.dram_tensor("cbv", [128, H * R * TPC], bf16, kind="ExternalInput")
    band_d = nc.dram_tensor("band", [128, band_tot], bf16, kind="ExternalInput")
    gd_d = {
        h: nc.dram_tensor(f"gdiag{h}", [DH, KS[h] * DH], bf16, kind="ExternalInput")
        for h in PE_STATIC_HEADS
    }
    gvec_d = nc.dram_tensor("gvec", [DH, H * 21], f32, kind="ExternalInput")
    b_in_d = nc.dram_tensor("b_in", [128, H], f32, kind="ExternalInput")
    ident_d = nc.dram_tensor("ident", [128, 128], bf16, kind="ExternalInput")
    out_d = nc.dram_tensor("out", [TPC, DM], bf16, kind="ExternalOutput")

    cbv4 = cbv_d.rearrange("p (hh r t) -> p hh r t", hh=H, r=R)
    NFC = H * NF * 128  # size of the "first" w block

    with tile.TileContext(nc) as tc:
        with tc.tile_pool(name="const", bufs=1) as pc:
            xp_sb = [pc.tile([DH, TH], bf16, name=f"xp{m}") for m in range(H)]
            o_sb = [pc.tile([DH, TPC], bf16, name=f"o{h}") for h in range(H)]
            ident_sb = pc.tile([128, 128], bf16, name="ident_sb")
            b_in_sb = pc.tile([128, H], f32, name="b_in_sb")
            gvec_sb = pc.tile([DH, H * 21], f32, name="gvec_sb")
            band_sb = pc.tile([128, band_tot], bf16, name="band_sb")
            gd_sb = {
                h: pc.tile([DH, KS[h] * DH], bf16, name=f"gd{h}")
                for h in PE_STATIC_HEADS
            }
            w_all = pc.tile([128, H * DM], bf16, name="w_all")
            x_all = pc.tile([128, H * TH], bf16, name="x_all")
            wo_sb = pc.tile([128, H * DM], bf16, name="wo_all")
            wo3 = wo_sb.rearrange("p (i m) -> p i m", i=H)

            # --- DMAs by criticality; every transfer is contiguous ---
            nc.sync.dma_start(out=w_all[:, :NFC], in_=wpk_d[:, :NFC])
            HS1 = H * S1C // 2
            nc.sync.dma_start(out=x_all[:, :HS1], in_=xpk_d[:, :HS1])
            nc.sync.dma_start(out=x_all[:, HS1 : H * S1C], in_=xpk_d[:, HS1 : H * S1C])
            nc.gpsimd.dma_start(out=ident_sb, in_=ident_d[:, :])
            nc.gpsimd.dma_start(out=b_in_sb, in_=b_in_d[:, :])
            nc.sync.dma_start(out=w_all[:, NFC:], in_=wpk_d[:, NFC:])
            nc.gpsimd.dma_start(out=band_sb, in_=band_d[:, :])
            # rest of x, split by stage-1 chunk so chunk 1 lands earlier
            nc.sync.dma_start(
                out=x_all[:, H * S1C : 2 * H * S1C],
                in_=xpk_d[:, H * S1C : 2 * H * S1C],
            )
            nc.sync.dma_start(
                out=x_all[:, 2 * H * S1C :], in_=xpk_d[:, 2 * H * S1C :]
            )
            for h in PE_STATIC_HEADS:
                nc.gpsimd.dma_start(out=gd_sb[h], in_=gd_d[h][:, :])
            nc.gpsimd.dma_start(out=gvec_sb, in_=gvec_d[:, :])

            # ---------------- stages 1+2 (head 7 overlaps stage 1) --------
            with tc.tile_pool(name="s2cbv", bufs=3) as p2c, tc.tile_pool(
                name="s2csv", bufs=3
            ) as p2v, tc.tile_pool(name="s2s", bufs=4) as p2s, tc.tile_pool(
                name="s2x", bufs=7
            ) as p2x, tc.tile_pool(name="s2acc", bufs=3) as p2a, tc.tile_pool(
                name="s2t", bufs=6
            ) as p2t:
                s2psum = tc.tile_pool(name="ps2tp", bufs=2, space="PSUM")
                pptp = s2psum.__enter__()
                s2psum_s = tc.tile_pool(name="ps2s", bufs=2, space="PSUM")
                ppss = s2psum_s.__enter__()
                s2psum_o = None
                ppso = None
                csv3_of = {}
                cbv_sb_of = {}
                xtd_of = {}

                def fetch_cbv(h, eng):
                    cbv_sb = p2c.tile([DH, R * TPC], bf16, name=f"cbv{h}", tag="cbv")
                    eng.dma_start(
                        out=cbv_sb.rearrange("p (r t) -> p r t", r=R),
                        in_=cbv4[:, h, :, :],
                    )
                    cbv_sb_of[h] = cbv_sb

                fetch_cbv(HEAD_ORDER[0], nc.sync)
                fetch_cbv(HEAD_ORDER[1], nc.sync)

                def transp_phase(h):
                    k, p, W, nw = _window_params(h)
                    xph = xp_sb[h]
                    xtds = []
                    for g0 in range(0, nw, 4):
                        gn = min(4, nw - g0)
                        tp4 = pptp.tile([128, 512], bf16, name="tp4", tag="tp4")
                        xtd4 = p2x.tile([128, 512], bf16, name="xtd4", tag="xtd4")
                        maxcnt = 0
                        for gi in range(gn):
                            b = g0 + gi
                            off = PAD - p + b * W
                            cnt = min(128, TH - off)
                            maxcnt = max(maxcnt, cnt)
                            nc.tensor.transpose(
                                tp4[:cnt, gi * 128 : gi * 128 + 128],
                                xph[:, off : off + cnt],
                                ident_sb,
                            )
                        nc.scalar.copy(
                            xtd4[:maxcnt, : gn * 128], tp4[:maxcnt, : gn * 128]
                        )
                        for gi in range(gn):
                            xtds.append((xtd4, gi))
                    xtd_of[h] = xtds

                def band_phase(h):
                    k, p, W, nw = _window_params(h)
                    bo = band_offs[h]
                    b2 = band_sb[:, bo : bo + R * W]
                    cbv3 = cbv_sb_of.pop(h).rearrange("p (r t) -> p r t", r=R)
                    csv_sb = p2v.tile([DH, R * TPC], bf16, name=f"csv{h}", tag="csv")
                    csv3 = csv_sb.rearrange("p (r t) -> p r t", r=R)
                    csv3_of[h] = csv3
                    xtds = xtd_of.pop(h)
                    for q0 in range(0, nw, 2):
                        qn = min(2, nw - q0)
                        # halves bank-aligned at 0/512 (a matmul output must
                        # not cross a 2KB PSUM bank boundary)
                        ps_s = ppss.tile([128, 1024], f32, name="ps_s", tag="ps_s")
                        s_sb = p2s.tile([128, 1024], bf16, name="s_sb", tag="s_sb")
                        muls = []
                        for qi in range(qn):
                            b = q0 + qi
                            off = PAD - p + b * W
                            cnt = min(128, TH - off)
                            n_out = min(W, TPC - b * W)
                            xtd4, gi = xtds[b]
                            st = xtd4[:cnt, gi * 128 : gi * 128 + 128]
                            if n_out == W:
                                nc.tensor.matmul(
                                    ps_s[:, qi * 512 : qi * 512 + R * W],
                                    st,
                                    b2[:cnt, :],
                                    start=True,
                                    stop=True,
                                )
                            else:
                                band3 = b2.rearrange("p (r w) -> p r w", r=R)
                                nc.tensor.matmul(
                                    ps_s[:, qi * 512 : qi * 512 + R * n_out],
                                    st,
                                    band3[:cnt, :, :n_out],
                                    start=True,
                                    stop=True,
                                )
                            muls.append((qi, b, n_out))
                        if h in PE_STATIC_HEADS:
                            # shorter chain into the PE diag stage: DVE
                            # multiplies straight from PSUM
                            for qi, b, n_out in muls:
                                t0 = b * W
                                nc.vector.tensor_mul(
                                    csv3[:, :, t0 : t0 + n_out],
                                    ps_s[
                                        :, qi * 512 : qi * 512 + R * n_out
                                    ].rearrange("p (r w) -> p r w", r=R),
                                    cbv3[:, :, t0 : t0 + n_out],
                                )
                        else:
                            # one Act evacuation for both halves, then bf16
                            # DVE multiplies (keeps DVE off the PSUM path)
                            hi_end = muls[-1][0] * 512 + R * muls[-1][2]
                            nc.scalar.copy(s_sb[:, :hi_end], ps_s[:, :hi_end])
                            for qi, b, n_out in muls:
                                t0 = b * W
                                nc.vector.tensor_mul(
                                    csv3[:, :, t0 : t0 + n_out],
                                    s_sb[
                                        :, qi * 512 : qi * 512 + R * n_out
                                    ].rearrange("p (r w) -> p r w", r=R),
                                    cbv3[:, :, t0 : t0 + n_out],
                                )

                def stage2b_chunks(h):
                    k, p, W, nw = _window_params(h)
                    xph = xp_sb[h]
                    npe = PE_SHIFTS.get(h, 0)
                    HC = TPC // 2
                    chunks = []
                    if npe > 0:
                        def pe_chunk(c0):
                            csv3 = csv3_of[h]
                            ps_o = ppso.tile([128, 512], f32, name="ps_o", tag="ps_o")
                            nmm = R + npe
                            idx = 0
                            for rr in range(R):
                                nc.tensor.matmul(
                                    ps_o,
                                    ident_sb,
                                    csv3[:, rr, c0 : c0 + 512],
                                    start=(idx == 0),
                                    stop=(idx == nmm - 1),
                                )
                                idx += 1
                            for j in range(npe):
                                o0 = c0 + j - p + PAD
                                nc.tensor.matmul(
                                    ps_o,
                                    gd_sb[h][:, j * DH : (j + 1) * DH],
                                    xph[:, o0 : o0 + 512],
                                    start=(idx == 0),
                                    stop=(idx == nmm - 1),
                                )
                                idx += 1
                            nc.scalar.copy(o_sb[h][:, c0 : c0 + 512], ps_o)
                        chunks += [lambda c0=c0: pe_chunk(c0) for c0 in (0, 512)]

                    if npe < k:
                        sacc = p2a.tile([DH, TPC], bf16, name="sacc", tag="sacc")

                        def dve_chunk(c0):
                            sl = slice(c0, c0 + HC)
                            for jj, j in enumerate(range(npe, k)):
                                sh = PAD + j - p + c0
                                gcol = gvec_sb[:, h * 21 + j : h * 21 + j + 1]
                                if jj == 0:
                                    nc.vector.tensor_scalar(
                                        out=sacc[:, sl],
                                        in0=xph[:, sh : sh + HC],
                                        scalar1=gcol,
                                        scalar2=None,
                                        op0=MUL,
                                    )
                                else:
                                    tmp = p2t.tile(
                                        [DH, HC], bf16, name="tmp", tag="tmp"
                                    )
                                    nc.vector.tensor_scalar(
                                        out=tmp,
                                        in0=xph[:, sh : sh + HC],
                                        scalar1=gcol,
                                        scalar2=None,
                                        op0=MUL,
                                    )
                                    nc.vector.tensor_add(
                                        sacc[:, sl], sacc[:, sl], tmp
                                    )
                            if npe > 0:
                                # PE already produced csv-sum + its shifts
                                nc.vector.tensor_add(
                                    o_sb[h][:, sl], o_sb[h][:, sl], sacc[:, sl]
                                )
                                return
                            csv3 = csv3_of[h]
                            t01 = p2t.tile([DH, HC], bf16, name="t01", tag="tmp")
                            nc.vector.tensor_add(t01, csv3[:, 0, sl], csv3[:, 1, sl])
                            t23 = p2t.tile([DH, HC], bf16, name="t23", tag="tmp")
                            nc.vector.tensor_add(t23, csv3[:, 2, sl], csv3[:, 3, sl])
                            nc.vector.tensor_add(t01, t01, t23)
                            nc.vector.tensor_add(o_sb[h][:, sl], t01, sacc[:, sl])

                        chunks += [lambda c0=c0: dve_chunk(c0) for c0 in (0, HC)]
                    return chunks

                # ---- stage 1 with head-7 stage-2a interleaved ----
                with tc.tile_pool(name="ps1", bufs=2, space="PSUM") as pp1:
                    for ci in range(3):
                        for mi, m in enumerate(HEAD_ORDER):
                            ps = pp1.tile([128, S1C], f32, name="ps_xp", tag="ps_xp")
                            for i in range(H):
                                nc.tensor.matmul(
                                    ps,
                                    w_all[:, _w_col(i, m) : _w_col(i, m) + 128],
                                    x_all[
                                        :, _x_col(i, ci) : _x_col(i, ci) + S1C
                                    ],
                                    start=(i == 0),
                                    stop=(i == H - 1),
                                )
                            nc.scalar.activation(
                                out=xp_sb[m][:, ci * S1C : (ci + 1) * S1C],
                                in_=ps,
                                func=mybir.ActivationFunctionType.Identity,
                                bias=b_in_sb[:, m : m + 1],
                                scale=1.0,
                            )
                            if ci == 2 and mi == 0:
                                transp_phase(HEAD_ORDER[0])
                            elif ci == 2 and mi == 2:
                                band_phase(HEAD_ORDER[0])
                                fetch_cbv(HEAD_ORDER[2], nc.gpsimd)

                s2psum_o = tc.tile_pool(name="ps2o", bufs=2, space="PSUM")
                ppso = s2psum_o.__enter__()

                pending = []
                deferred_b = []
                for fn in stage2b_chunks(HEAD_ORDER[0]):
                    pending.append(fn)
                transp_phase(HEAD_ORDER[1])
                for hi in range(1, H):
                    h = HEAD_ORDER[hi]
                    band_phase(h)
                    if hi + 1 < H:
                        hn = HEAD_ORDER[hi + 1]
                        if hi + 2 < H:
                            fetch_cbv(
                                HEAD_ORDER[hi + 2],
                                nc.sync if hi % 2 == 0 else nc.gpsimd,
                            )
                        transp_phase(hn)
                    if hi == 2:
                        nc.gpsimd.dma_start(out=wo_sb, in_=wopk_d[:, :])
                    for fn in stage2b_chunks(h):
                        if h in PASS_B:
                            deferred_b.append(fn)
                        else:
                            pending.append(fn)
                    for _ in range(2):
                        if pending:
                            pending.pop(0)()
                for fn in pending:
                    fn()
                # stage-2 PSUM pools closed; PASS_B heads' DVE statics still
                # to run (no PSUM needed) - they overlap out-proj pass A
                s2psum_o.__exit__(None, None, None)
                s2psum_s.__exit__(None, None, None)
                s2psum.__exit__(None, None, None)

                # ---------------- stage 4: out projection ----------------
                PASS_A = [i for i in HEAD_ORDER if i not in PASS_B]
                part_sb = [
                    pc.tile([128, DM], bf16, name=f"part{t}")
                    for t in range(TPC // 128)
                ]
                with tc.tile_pool(name="s4o", bufs=3) as p4o, tc.tile_pool(
                    name="ps4", bufs=4, space="PSUM"
                ) as pp4:
                    # pass A: heads done early; runs while DVE finishes the
                    # last two heads' static convs
                    for t in range(TPC // 128):
                        for e0 in (0, 512):
                            ps = pp4.tile(
                                [128, 512], f32, name="ps_out", tag="ps_out"
                            )
                            for ii, i in enumerate(PASS_A):
                                nc.tensor.matmul(
                                    ps,
                                    o_sb[i][:, t * 128 : (t + 1) * 128],
                                    wo3[:, i, e0 : e0 + 512],
                                    start=(ii == 0),
                                    stop=(ii == len(PASS_A) - 1),
                                )
                            nc.scalar.copy(part_sb[t][:, e0 : e0 + 512], ps)
                    for fn in deferred_b:
                        fn()
                    # pass B: late heads' contribution + DVE add of partial
                    for t in range(TPC // 128):
                        ot = p4o.tile([128, DM], bf16, name="out_sb", tag="out_sb")
                        for e0 in (0, 512):
                            ps = pp4.tile(
                                [128, 512], f32, name="ps_out", tag="ps_out"
                            )
                            for ii, i in enumerate(PASS_B):
                                nc.tensor.matmul(
                                    ps,
                                    o_sb[i][:, t * 128 : (t + 1) * 128],
                                    wo3[:, i, e0 : e0 + 512],
                                    start=(ii == 0),
                                    stop=(ii == len(PASS_B) - 1),
                                )
                            nc.vector.tensor_add(
                                ot[:, e0 : e0 + 512],
                                ps,
                                part_sb[t][:, e0 : e0 + 512],
                            )
                        eng = nc.gpsimd if (t % 2 == 0) else nc.sync
                        eng.dma_start(out=out_d[t * 128 : (t + 1) * 128, :], in_=ot)

    _split_multi_waits(nc, mybir)
    return nc


def _host_prep(inputs):
    x = np.ascontiguousarray(np.asarray(inputs["x"], dtype=np.float32))
    W_in = np.asarray(inputs["W_in"], dtype=np.float32)
    b_in = np.asarray(inputs["b_in"], dtype=np.float32)
    W_out = np.asarray(inputs["W_out"], dtype=np.float32)
    b_out = np.asarray(inputs["b_out"], dtype=np.float32)
    Wc = np.asarray(inputs["Wc"], dtype=np.float32)
    A = np.asarray(inputs["A"], dtype=np.float32)
    V = np.asarray(inputs["V"], dtype=np.float32)
    base = np.asarray(inputs["base"], dtype=np.float32)
    alphas = np.asarray(inputs["alphas"], dtype=np.float32)

    alpha = 1.0 / (1.0 + np.exp(-alphas))

    W_inT = np.ascontiguousarray(W_in.T)
    W_outT = np.ascontiguousarray(W_out.T)
    Wc_aug = np.zeros((DM, H * R), dtype=np.float32)
    c_bias = np.zeros((H * R,), dtype=np.float32)
    for h in range(H):
        Wc_aug[:, R * h : R * h + R] = W_inT[:, h * DH : (h + 1) * DH] @ Wc[h]
        c_bias[R * h : R * h + R] = b_in[h * DH : (h + 1) * DH] @ Wc[h]

    # pack W_in into the SBUF layout (see _w_col)
    wpk = np.zeros((128, H * DM), dtype=np.float32)
    for i in range(H):
        for m in range(H):
            wpk[:, _w_col(i, m) : _w_col(i, m) + 128] = W_inT[
                i * 128 : (i + 1) * 128, m * 128 : (m + 1) * 128
            ]
    wopk = np.zeros((128, H * DM), dtype=np.float32)
    for i in range(H):
        wopk[:, i * DM : (i + 1) * DM] = W_outT[i * 128 : (i + 1) * 128, :]

    band_offs, band_tot = _band_offsets()
    band_all = np.zeros((128, band_tot), dtype=np.float32)
    for h in range(H):
        k, p, W, nw = _window_params(h)
        t_in = np.arange(128)[:, None]
        t_out = np.arange(W)[None, :]
        delta = t_in - t_out
        mask = (delta >= 0) & (delta < k)
        dc = np.clip(delta, 0, k - 1)
        band = np.zeros((128, R, W), dtype=np.float32)
        for rr in range(R):
            band[:, rr, :] = np.where(mask, A[h, rr][dc], 0.0)
        band_all[:, band_offs[h] : band_offs[h] + R * W] = band.reshape(128, R * W)

    prep = {
        "wpk": wpk.astype(BF),
        "wopk": wopk.astype(BF),
        "b_in": np.ascontiguousarray(b_in.reshape(H, 128).T),
        "ident": np.eye(128, dtype=BF),
        "band": band_all.astype(BF),
    }

    for h in PE_STATIC_HEADS:
        k = KS[h]
        gd = np.zeros((DH, k, DH), dtype=np.float32)
        g = (1.0 - alpha[h]) * base[h, :k]
        dd = np.arange(DH)
        gd[dd, :, dd] = g.T[dd]
        prep[f"gdiag{h}"] = gd.reshape(DH, k * DH).astype(BF)

    gvec = np.zeros((DH, H, 21), dtype=np.float32)
    for h in range(H):
        k = KS[h]
        gvec[:, h, :k] = ((1.0 - alpha[h]) * base[h, :k]).T
    prep["gvec"] = gvec.reshape(DH, H * 21).copy()

    xT_slices = []
    cbv_slices = []
    per_b = NC // B
    for c in range(NC):
        bb = c // per_b
        s = (c % per_b) * TPC
        sl = np.zeros((TH, DM), dtype=np.float32)
        lo, hi = s - PAD, s + TPC + PAD
        clo, chi = max(lo, 0), min(hi, N)
        sl[clo - lo : chi - lo] = x[bb, clo:chi]
        slT = sl.T  # (DM, TH)
        xpk = np.empty((128, H * TH), dtype=np.float32)
        for i in range(H):
            blk = slT[i * 128 : (i + 1) * 128]
            for ci in range(3):
                xpk[:, _x_col(i, ci) : _x_col(i, ci) + S1C] = blk[
                    :, ci * S1C : (ci + 1) * S1C
                ]
        xT_slices.append(xpk.astype(BF))
        cc = sl[PAD : PAD + TPC] @ Wc_aug + c_bias[None, :]
        cc3 = cc.reshape(TPC, H, R)
        cbv = np.empty((128, H, R, TPC), dtype=np.float32)
        for h in range(H):
            for rr in range(R):
                cbv[:, h, rr, :] = alpha[h] * np.outer(V[h, rr], cc3[:, h, rr])
        cbv_slices.append(
            np.ascontiguousarray(cbv.reshape(128, H * R * TPC)).astype(BF)
        )
    return prep, xT_slices, cbv_slices, b_out


def _run(inputs, trace=False, **kwargs):
    _install_ntff_hook_shim()
    from concourse.bass_utils import run_bass_kernel_spmd

    if "mod" not in _MODULE_CACHE:
        _MODULE_CACHE["mod"] = _build_module()
    nc = _MODULE_CACHE["mod"]

    prep, xT_slices, cbv_slices, b_out = _host_prep(inputs)
    in_maps = []
    for c in range(NC):
        m = dict(prep)
        m["xpk"] = xT_slices[c]
        m["cbv"] = cbv_slices[c]
        in_maps.append(m)

    res = run_bass_kernel_spmd(
        nc, in_maps, core_ids=list(range(NC)), trace=trace, **kwargs
    )
    outs = [np.asarray(res.results[c]["out"], dtype=np.float32) for c in range(NC)]
    full = np.concatenate(outs, axis=0).reshape(B, N, DM)
    full += b_out[None, None, :]
    return full, res


def kernel(**inputs) -> np.ndarray:
    return _run(inputs)[0]
